# revision 105
# baseline (speedup 1.0000x reference)
"""DeltaNet forward Trainium2 kernel (8-core SPMD, batch x head-pair sharded).

Sharding: core c -> batch b=c//2, head-group hg=c%2 (heads 2hg, 2hg+1 = columns
hg*512 : hg*512+512 of the q/k/v/g projections).  Each core computes a partial
[L, D] output (its two heads' contribution through the output projection); the
host sums the two bf16 partials per batch in fp32.  norm_w is folded into Wo
on the host; weights and x are fed to the device in bf16 (host-converted).

Per-core pipeline, software-pipelined across 8 strips of 512 tokens:
  prep(s):  x^T strip DMA (one 1MB transfer) -> q/k/v projections
            z^T = W^T x^T (bf16 matmuls) -> depthwise causal conv = 4
            accumulating diag(w_tap) matmuls reusing the projection's PSUM
            bank -> SiLU; g projection (token-major) with the swish gate
            fused into a single SiLU (o*g*sigmoid(g) = o*silu(g)); all four
            chunks' betas in one batched sigmoid; l2-norm scales via
            ones-matmul column sums + PE row-broadcast; k-hat/q-hat scaled
            copies; token-major k-hat/v via two DMA-xbar transposes each.
  scan(s):  per 128-token chunk, delta-rule scan in bf16; (I+A)^-1 by a
            2-level Neumann product form (A is strongly contractive here);
            A^T/attn^T/A masked+scaled in one 3-part-mask DVE op; only -T is
            materialized (kbtok and U_t carry the compensating signs);
            multi-region PSUM tiles cut the per-chunk allocation count so
            consecutive chunks' ladders overlap; S in fp32 master + bf16
            working copy.
  flush(s): batched RMS-norm + swish gate, one DMA-xbar transpose per chunk,
            output projection -> bf16 partial out [L, D] (4-deep staging so
            the pipe is not paced by out-DMA completion).
  Emission interleaves prep(s+1) into scan(s) chunk-by-chunk; the Tile list
  scheduler uses emission order as priority, so this paces PE-dense prep work
  into the latency-bound scan chain.  Activation-table loads are kept to
  ~3/strip by clustering SiLU / sigmoid / abs-rsqrt uses.
"""

import sys

sys.path.insert(0, "/opt/trn_rl_repo")

from contextlib import ExitStack

import numpy as np

import concourse.bass as bass
import concourse.tile as tile
from concourse import bacc, mybir
from concourse.bass_utils import run_bass_kernel_spmd
from concourse.masks import make_identity

FP = mybir.dt.float32
BF = mybir.dt.bfloat16
AF = mybir.ActivationFunctionType
OP = mybir.AluOpType

B, L, D, H = 4, 4096, 1024, 4
Dh = 256          # head dim
DL = 512          # per-core channels (2 heads)
KT = 4            # conv taps
C = 128           # chunk length
LT = 512          # L-strip size
NS = L // LT      # 8 strips
CPS = LT // C     # 4 chunks per strip
NLEV = 2          # Neumann levels (A is strongly contractive, A^8 ~ 1e-3)
EPS_RMS = 1e-5
EPS_L2 = 1e-12


def deltanet_core(ctx: ExitStack, tc: tile.TileContext, io: dict):
    nc = tc.nc
    x, wq, wk, wv, wg, wb, wo, cq, ck, cv, out = (
        io["x"], io["wq"], io["wk"], io["wv"], io["wg"], io["wb"], io["wo"],
        io["cq"], io["ck"], io["cv"], io["out"])

    pool1 = ctx.enter_context(tc.tile_pool(name="consts", bufs=1))
    xpool = ctx.enter_context(tc.tile_pool(name="xp", bufs=2))
    zpool = ctx.enter_context(tc.tile_pool(name="zp", bufs=3))
    strip = ctx.enter_context(tc.tile_pool(name="strip", bufs=2))
    small = ctx.enter_context(tc.tile_pool(name="small", bufs=2))
    hot = ctx.enter_context(tc.tile_pool(name="hot", bufs=5))
    spool = ctx.enter_context(tc.tile_pool(name="state", bufs=1))
    psA = ctx.enter_context(tc.tile_pool(name="psA", bufs=3, space="PSUM"))

    psS = ctx.enter_context(tc.tile_pool(name="psS", bufs=4, space="PSUM"))
    psT = ctx.enter_context(tc.tile_pool(name="psT", bufs=1, space="PSUM"))



    # ---- first x strip + resident constants (DMA-ordered by first use) -----
    xr0 = x.rearrange("(t p) l -> p t l", p=128)
    xt0 = xpool.tile([128, 8, LT], BF, tag="xt", name="xt0")
    wt = {}
    wt["q"] = pool1.tile([128, 8, DL], BF, tag="wq", name="wq_t")
    wqr = wq.rearrange("(t p) n -> p t n", p=128)
    for quar in range(4):
        hs = slice(2 * quar, 2 * quar + 2)
        nc.sync.dma_start(out=xt0[:, hs, :], in_=xr0[:, hs, 0:LT])
        nc.sync.dma_start(wt["q"][:, hs, :], wqr[:, hs, :])
    for name, w in (("k", wk), ("v", wv)):
        t = pool1.tile([128, 8, DL], BF, tag=f"w{name}", name=f"w{name}")
        nc.sync.dma_start(t[:], w.rearrange("(t p) n -> p t n", p=128))
        wt[name] = t
    cw = {}
    for name, cz in (("q", cq), ("k", ck), ("v", cv)):
        t = pool1.tile([128, 4, KT], FP, tag=f"c{name}", name=f"c{name}")
        nc.sync.dma_start(t[:], cz.rearrange("(t p) j -> p t j", p=128))
        cw[name] = t
    wb_t = pool1.tile([128, 8, 2], BF, tag="wb")
    nc.sync.dma_start(wb_t[:], wb.rearrange("(t p) n -> p t n", p=128))
    t = pool1.tile([128, 8, DL], BF, tag="wg", name="wg_t")
    nc.sync.dma_start(t[:], wg.rearrange("(t p) n -> p t n", p=128))
    wt["g"] = t
    wo_t = pool1.tile([128, 4, 1024], BF, tag="wo")
    nc.sync.dma_start(wo_t[:], wo.rearrange("(t p) n -> p t n", p=128))

    # mask_ua[:, 0, :] strict-upper, mask_ua[:, 1, :] inclusive-upper
    mask3 = pool1.tile([128, 3, 128], BF, tag="mask3")
    nc.gpsimd.memset(mask3, 1.0)
    # parts 0,1: keep where y + a - x > 0 (a=0 strict-upper, a=1 incl-upper)
    nc.gpsimd.affine_select(out=mask3[:, 0:2, :], in_=mask3[:, 0:2, :],
                            compare_op=OP.is_gt, fill=0.0, base=0,
                            pattern=[[1, 2], [1, 128]], channel_multiplier=-1)
    # part 2: strict-lower
    nc.gpsimd.affine_select(out=mask3[:, 2, :], in_=mask3[:, 2, :],
                            compare_op=OP.is_gt, fill=0.0, base=0,
                            pattern=[[-1, 128]], channel_multiplier=1)
    mask_ua = mask3[:, 0:2, :]
    ident_f = pool1.tile([128, 128], FP, tag="identf")
    ident_b = pool1.tile([128, 128], BF, tag="identb")
    make_identity(nc, ident_f)
    make_identity(nc, ident_b)

    ones_col = pool1.tile([128, 1], BF, tag="ones_col")
    nc.vector.memset(ones_col, 1.0)
    # sel24[:, r, :] is e_r (x) ones: lhsT for K=24 row-broadcast matmuls
    sel24 = pool1.tile([24, 24, 128], BF, tag="sel24")
    nc.gpsimd.memset(sel24, 1.0)
    nc.gpsimd.affine_select(out=sel24[:], in_=sel24[:], compare_op=OP.is_equal,
                            fill=0.0, base=0, pattern=[[-1, 24], [0, 128]],
                            channel_multiplier=1)
    eps_l2 = pool1.tile([128, 1], FP, tag="epsl2")
    nc.vector.memset(eps_l2, EPS_L2 * EPS_L2)
    eps_rms = pool1.tile([128, 1], FP, tag="epsrms")
    nc.vector.memset(eps_rms, EPS_RMS)
    sq_scr = pool1.tile([128, 256], BF, tag="sq_scr")

    # conv diag tiles (bf16)
    diag = {}
    for name in ("q", "k", "v"):
        dt = pool1.tile([128, 4, KT, 128], BF, tag=f"diag{name}")
        diag[name] = dt
        for ct in range(4):
            for j in range(KT):
                nc.vector.tensor_scalar_mul(dt[:, ct, j, :], ident_b[:],
                                            cw[name][:, ct, j:j + 1])

    # ---- persistent state ---------------------------------------------------
    S32 = [spool.tile([128, 2, 256], FP, name=f"S32_{h}", tag=f"S32_{h}")
           for h in range(2)]
    Sbf = [spool.tile([128, 2, 256], BF, name=f"Sbf_{h}", tag=f"Sbf_{h}")
           for h in range(2)]
    for h in range(2):
        nc.vector.memset(S32[h], 0.0)
        nc.vector.memset(Sbf[h], 0.0)
    carries = {}
    for name in ("q", "k", "v"):
        for ct in range(4):
            cr = spool.tile([128, KT - 1], BF, tag=f"carry_{name}{ct}",
                            name="cr")
            nc.vector.memset(cr, 0.0)
            carries[(name, ct)] = cr

    xr = x.rearrange("(t p) l -> p t l", p=128)


    # ---- per-strip emission helpers ----------------------------------------
    def make_prep(s, xt_pre=None):
        """Returns (P, main_thunks, tail_thunks) for strip s."""
        P = {}
        l0 = s * LT

        def t_x():
            if xt_pre is not None:
                xt = xt_pre
            else:
                xt = xpool.tile([128, 8, LT], BF, tag="xt", name="xt")
                nc.sync.dma_start(out=xt[:], in_=xr[:, :, l0:l0 + LT])
            P["xt"] = xt
            P["ssqs"] = small.tile([128, CPS, 2], FP, tag="ssqs", name="ssqs")
            P["vtok"] = strip.tile([128, 2, 2, CPS, 128], BF, tag="vtok",
                                   name="vtok")
            P["ktok"] = strip.tile([128, 2, 2, CPS, 128], BF, tag="ktok",
                                   name="ktok")
            P["gg"] = strip.tile([128, CPS, DL], BF, tag="gg", name="gg")
            P["osb"] = strip.tile([128, CPS, 2, 256], BF, tag="osb",
                                  name="osb")

        def mk_qkv(name, ct):
            def f():
                xt = P["xt"]
                zp_ = psA.tile([128, LT], FP, tag="mm512", name="zp")
                for kt8 in range(8):
                    nc.tensor.matmul(
                        zp_[:], wt[name][:, kt8, bass.ts(ct, 128)],
                        xt[:, kt8, :], start=(kt8 == 0), stop=(kt8 == 7))
                ze = zpool.tile([128, KT - 1 + LT], BF, tag="zext", name="ze")
                nc.gpsimd.tensor_copy(ze[:, 0:KT - 1], carries[(name, ct)][:])
                nc.any.tensor_copy(ze[:, KT - 1:], zp_[:])
                nc.gpsimd.tensor_copy(carries[(name, ct)][:],
                                      ze[:, LT:LT + KT - 1])
                zc = zp_
                for j in range(KT):
                    nc.tensor.matmul(zc[:], diag[name][:, ct, j, :],
                                     ze[:, j:j + LT],
                                     start=(j == 0), stop=(j == KT - 1))
                if name == "v":
                    h, ct2 = divmod(ct, 2)
                    if ("zsv", h) not in P:
                        P[("zsv", h)] = strip.tile([128, 2, LT], BF,
                                                   tag=f"zsv{h}", bufs=1,
                                                   name="zsv")
                    zst = P[("zsv", h)][:, ct2, :]
                    nc.scalar.activation(zst, zc[:], AF.Silu)
                    if ct2 == 1:
                        nc.sync.dma_start_transpose(P["vtok"][:, h],
                                                    P[("zsv", h)][:])
                else:
                    zst = strip.tile([128, LT], BF, tag=f"zs_{name}{ct}",
                                     bufs=1, name="zst")
                    nc.scalar.activation(zst[:], zc[:], AF.Silu)
                    P[("zs", name, ct)] = zst
                    sqt = strip.tile([128, LT], BF, tag=f"sq_{name}{ct}",
                                     bufs=1, name="sqt")
                    nc.gpsimd.tensor_tensor(sqt[:], zst[:], zst[:],
                                            op=OP.mult)
                    P[("sq", name, ct)] = sqt
            return f

        def mk_g(lt):
            def f():
                xt = P["xt"]
                gp = psA.tile([128, 512], FP, tag="mm512", name="gp")
                for kt8 in range(8):
                    nc.tensor.matmul(gp[:], xt[:, kt8, bass.ts(lt, 128)],
                                     wt["g"][:, kt8, :],
                                     start=(kt8 == 0), stop=(kt8 == 7))
                nc.scalar.activation(P["gg"][:, lt, :], gp[:], AF.Silu)
            return f

        def t_beta():
            xt = P["xt"]
            bp = psT.tile([128, 512], FP, tag="tiny", name="bp")
            for lt in range(CPS):
                for kt8 in range(8):
                    nc.tensor.matmul(bp[:, 2 * lt:2 * lt + 2],
                                     xt[:, kt8, bass.ts(lt, 128)],
                                     wb_t[:, kt8, :],
                                     start=(kt8 == 0), stop=(kt8 == 7))
            betas = small.tile([128, CPS, 2], FP, tag="betas", name="betas")
            nc.scalar.activation(betas[:].rearrange("p l h -> p (l h)"),
                                 bp[:, 0:8], AF.Sigmoid)
            P["betas"] = betas

        def t_psq():
            # l2 column sums + rsq scales; rsqall cols: ni*8+h*4+lt for
            # rsq_{q,k}; 16+h*4+lt for rsq_k * beta
            psq = psT.tile([128, 512], FP, tag="tiny", name="psq")
            for ni, name in enumerate(("q", "k")):
                for h in range(2):
                    for lt in range(CPS):
                        col = ni * 8 + h * 4 + lt
                        for ct2 in range(2):
                            nc.tensor.matmul(
                                psq[:, col:col + 1],
                                P[("sq", name, 2 * h + ct2)][:,
                                                             bass.ts(lt, 128)],
                                ones_col[:], start=(ct2 == 0),
                                stop=(ct2 == 1))
            rsqall = small.tile([128, 24], FP, tag="rsqall", name="rsqall")
            nc.scalar.activation(rsqall[:, 0:16], psq[:, 0:16],
                                 AF.Abs_reciprocal_sqrt, bias=eps_l2[:])
            nc.vector.tensor_tensor(
                rsqall[:, 16:24].rearrange("p (h l) -> p h l", h=2),
                rsqall[:, 8:16].rearrange("p (h l) -> p h l", h=2),
                P["betas"].rearrange("p l h -> p h l"), op=OP.mult)
            rpt = psq
            nc.tensor.matmul(rpt[0:24, 128:256], rsqall[:], ident_f[:],
                             is_transpose=True, start=True, stop=True)
            rr24 = small.tile([24, 128], BF, tag="rr24", name="rr24")
            nc.scalar.copy(rr24[:], rpt[0:24, 128:256])
            P["rr24"] = rr24

        def mk_bcast(h):
            def f():
                bc = strip.tile([128, CPS, 3, 128], BF, tag=f"bcast{h}",
                                bufs=1, name="bc")
                for lt in range(CPS):
                    bp2 = psA.tile([128, 3 * 128], FP, tag="mm512",
                                   name="bp2")
                    for r, row in enumerate((h * 4 + lt, 8 + h * 4 + lt,
                                             16 + h * 4 + lt)):
                        nc.tensor.matmul(bp2[:, bass.ts(r, 128)],
                                         sel24[:, row, :], P["rr24"][:],
                                         start=True, stop=True)
                    nc.any.tensor_copy(bc[:, lt, :, :], bp2[:])
                P[("bc", h)] = bc
            return f

        def mk_kqkh(h, ct2):
            def f():
                ct = 2 * h + ct2
                bc = P[("bc", h)]

                def c4(ap):
                    return ap.rearrange("p (a b) -> p a b", a=CPS)

                kq = strip.tile([128, 2, LT], BF, tag=f"kqT{h}{ct2}",
                                name="kq")
                nc.vector.tensor_tensor(c4(kq[:, 0, :]),
                                        c4(P[("zs", "k", ct)][:]),
                                        bc[:, :, 2, :], op=OP.mult)
                nc.vector.tensor_tensor(c4(kq[:, 1, :]),
                                        c4(P[("zs", "q", ct)][:]),
                                        bc[:, :, 0, :], op=OP.mult)
                if ("khT", h) not in P:
                    P[("khT", h)] = strip.tile([128, 2, LT], BF,
                                               tag=f"khT{h}", name="khh")
                kh = P[("khT", h)][:, ct2, :]
                nc.vector.tensor_tensor(c4(kh), c4(P[("zs", "k", ct)][:]),
                                        bc[:, :, 1, :], op=OP.mult)
                P[("kqT", h, ct2)] = kq
                if ct2 == 1:
                    nc.sync.dma_start_transpose(P["ktok"][:, h],
                                                P[("khT", h)][:])
            return f

        main = [t_x, t_beta]
        for name in ("q", "k", "v"):
            for ct in range(4):
                main.append(mk_qkv(name, ct))
        tail = [t_psq, mk_bcast(0), mk_bcast(1)]
        for h in range(2):
            for ct2 in range(2):
                tail.append(mk_kqkh(h, ct2))
        for lt in range(CPS):
            tail.append(mk_g(lt))
        return P, main, tail

    def scan_chunk(P, lt, merge_ps=False, s_mode=0):
        betas = P["betas"]
        ktok, vtok = P["ktok"], P["vtok"]
        cs = bass.ts(lt, 128)
        # The two heads' scans are emitted stage-interleaved: engines execute
        # their streams in order, so alternating heads gives each head's
        # mm->copy->mm ladder a gap-filler.
        vb, kbtok, ATat, Alow = {}, {}, {}, {}
        R, Lk, Uk, negT, Ut, pos = {}, {}, {}, {}, {}, {}
        p1, p2, pp, pw, pu = {}, {}, {}, {}, {}
        for h in range(2):
            bcol = betas[:, lt, h:h + 1]
            vb[h] = hot.tile([128, 2, 128], BF, tag="vb", name=f"vb{h}")
            nc.gpsimd.tensor_scalar_mul(vb[h][:], vtok[:, h, :, lt, :], bcol)
            kbtok[h] = hot.tile([128, 2, 128], BF, tag="kbtok",
                                name=f"kbtok{h}")
            nc.gpsimd.tensor_scalar(kbtok[h][:], ktok[:, h, :, lt, :],
                                    bcol, -1.0, op0=OP.mult, op1=OP.mult)
        for h in range(2):
            # T1 regions: p1 (A^T|attn^T) 0:256, p2 (A) 256:384,
            # lev-1 R-update 384:512
            p1[h] = psS.tile([128, 512], FP, tag="scan", name="p1")
            for ct2 in range(2):
                nc.tensor.matmul(p1[h][:, 0:256],
                                 P[("khT", h)][:, ct2, cs],
                                 P[("kqT", h, ct2)][:, :, cs],
                                 start=(ct2 == 0), stop=(ct2 == 1))
            p2[h] = p1[h]
            for ct2 in range(2):
                nc.tensor.matmul(p2[h][:, 256:384],
                                 P[("kqT", h, ct2)][:, 0, cs],
                                 P[("khT", h)][:, ct2, cs],
                                 start=(ct2 == 0), stop=(ct2 == 1))
        for h in range(2):
            AAl = hot.tile([128, 384], BF, tag="ATat", name=f"AAl{h}")
            nc.vector.tensor_tensor(
                AAl[:].rearrange("p (a b) -> p a b", a=3),
                p1[h][:, 0:384].rearrange("p (a b) -> p a b", a=3),
                mask3[:], op=OP.mult)
            ATat[h] = AAl[:, 0:256]
            Alow[h] = AAl[:, 256:384]
            R[h] = hot.tile([128, 128], BF, tag="Rn", name=f"R{h}")
            nc.gpsimd.tensor_tensor(R[h][:], ident_b[:], AAl[:, 0:128],
                                    op=OP.subtract)
            Lk[h] = Alow[h]
            Uk[h] = ATat[h][:, 0:128]
        # Neumann (NLEV=2): (I - A_T)(I + A_T^2)(I + A_T^4).
        # T2 regions: lev0 UL/LU 0:256, lev1 UL/LU 256:512, pw -> 0:128
        # (reused after lev0 copy); lev1 R-update -> T1 384:512.
        for h in range(2):
            pp[h] = psS.tile([128, 512], FP, tag="scan", name="pp")
            nc.tensor.matmul(pp[h][:, 0:128], Uk[h], Lk[h],
                             start=True, stop=True)
            nc.tensor.matmul(pp[h][:, 128:256], Lk[h], Uk[h],
                             start=True, stop=True)
        for h in range(2):
            LUR = hot.tile([128, 384], BF, tag="LUR", name=f"LUR{h}")
            nc.any.tensor_copy(LUR[:, 0:256], pp[h][:, 0:256])
            Lk[h] = LUR[:, 0:128]
            Uk[h] = LUR[:, 128:256]
        for h in range(2):
            nc.tensor.matmul(pp[h][:, 256:384], Uk[h], Lk[h],
                             start=True, stop=True)
            nc.tensor.matmul(pp[h][:, 384:512], Lk[h], Uk[h],
                             start=True, stop=True)
            nc.tensor.matmul(p1[h][:, 384:512], Lk[h], R[h][:],
                             start=True, stop=False)
            nc.tensor.matmul(p1[h][:, 384:512], ident_b[:], R[h][:],
                             start=False, stop=True)
        for h in range(2):
            LUR = hot.tile([128, 384], BF, tag="LUR", name=f"LUR{h}b")
            nc.any.tensor_copy(LUR[:, 0:256], pp[h][:, 256:512])
            nc.any.tensor_copy(LUR[:, 256:384], p1[h][:, 384:512])
            R[h] = LUR[:, 256:384]
            Lk[h] = LUR[:, 0:128]
            Uk[h] = LUR[:, 128:256]
        for h in range(2):
            # final factor: R <- (I + A_T^4) R, into T2 0:128 (freed)
            pw[h] = pp[h]
            nc.tensor.matmul(pw[h][:, 0:128], Lk[h], R[h][:],
                             start=True, stop=False)
            nc.tensor.matmul(pw[h][:, 0:128], ident_b[:], R[h][:],
                             start=False, stop=True)
        for h in range(2):
            negT[h] = hot.tile([128, 128], BF, tag="negT", name=f"negT{h}")
            nc.vector.tensor_scalar_mul(negT[h][:], pw[h][:, 0:128], -1.0)
        for h in range(2):
            pu[h] = psS.tile([128, 512], FP, tag="scan", name="pu")
            for half in range(2):
                nc.tensor.matmul(pu[h][:, bass.ts(half, 128)],
                                 kbtok[h][:, half, :], negT[h][:],
                                 start=True, stop=True)
        WT = {}
        for h in range(2):
            WT[h] = hot.tile([128, 2, 128], BF, tag="WT", name=f"WT{h}")
            nc.any.tensor_copy(WT[h][:], pu[h][:, 0:256])
        for h in range(2):
            # -U = (-T) vb + W S (accumulated in psum); Ut negates on copy
            nc.tensor.matmul(pu[h][:, 256:512], negT[h][:],
                             vb[h][:].rearrange("p a b -> p (a b)"),
                             start=True, stop=False)
            for half in range(2):
                nc.tensor.matmul(pu[h][:, 256:512], WT[h][:, half, :],
                                 Sbf[h][:, half, :],
                                 start=False, stop=(half == 1))
        for h in range(2):
            Ut[h] = hot.tile([128, 256], BF, tag="Ut", name=f"Ut{h}")
            nc.vector.tensor_scalar_mul(Ut[h][:], pu[h][:, 256:512], -1.0)
        for h in range(2):
            if merge_ps:
                po = pp[h]
            else:
                po = psS.tile([128, 512], FP, tag="scan", name="po")
            for half in range(2):
                nc.tensor.matmul(po[:, 0:256], P[("kqT", h, half)][:, 1, cs],
                                 Sbf[h][:, half, :],
                                 start=(half == 0), stop=False)
            nc.tensor.matmul(po[:, 0:256], ATat[h][:, 128:256], Ut[h][:],
                             start=False, stop=True)
            pos[h] = po
            if s_mode != 2:
                if merge_ps:
                    psu = p1[h]
                else:
                    psu = psS.tile([128, 512], FP, tag="scan", name="psu")
                for half in range(2):
                    nc.tensor.matmul(psu[:, bass.ts(half, 256)],
                                     ktok[:, h, half, lt, :], Ut[h][:],
                                     start=True, stop=True)
            if s_mode == 0:
                nc.vector.tensor_tensor(
                    S32[h][:].rearrange("p a b -> p (a b)"), psu[:, 0:512],
                    S32[h][:].rearrange("p a b -> p (a b)"), op=OP.add)
                nc.gpsimd.tensor_copy(Sbf[h][:], S32[h][:])
            elif s_mode == 1:
                # tail strip: Sbf straight from psu + S32_old (short chain);
                # fp32 master updated in parallel for the next chunk's read
                nc.vector.scalar_tensor_tensor(
                    Sbf[h][:].rearrange("p a b -> p (a b)"), psu[:, 0:512],
                    1.0, S32[h][:].rearrange("p a b -> p (a b)"),
                    op0=OP.mult, op1=OP.add)
                nc.vector.tensor_tensor(
                    S32[h][:].rearrange("p a b -> p (a b)"), psu[:, 0:512],
                    S32[h][:].rearrange("p a b -> p (a b)"), op=OP.add)
            # o to SBUF + sum of squares for the batched RMS norm
            if phase == "stail":
                P[("po", lt, h)] = po
            else:
                nc.any.tensor_copy(P["osb"][:, lt, h, :], po[:, 0:256])
            nc.scalar.activation(sq_scr[:], po[:, 0:256], AF.Square,
                                 accum_out=P["ssqs"][:, lt, h:h + 1])

    def flush(P, s, lts=None, rv=None, pe_transpose=False):
        l0 = s * LT
        lts = range(CPS) if lts is None else lts
        if rv is None:
            rv = small.tile([128, CPS, 2], FP, tag="rv", bufs=1, name="rv")
            nc.scalar.activation(rv[:].rearrange("p a b -> p (a b)"),
                                 P["ssqs"][:].rearrange("p a b -> p (a b)"),
                                 AF.Abs_reciprocal_sqrt, bias=eps_rms[:],
                                 scale=1.0 / Dh)
        if "otT" not in P:
            P["otT"] = strip.tile([128, CPS, 4, 128], BF, tag="otT",
                                  name="otT")
        otT = P["otT"]
        for lt in lts:
            ogh = hot.tile([128, 512], BF, tag="ogh", bufs=3, name="ogh")
            for h in range(2):
                src_o = (P[("po", lt, h)][:, 0:256] if pe_transpose
                         else P["osb"][:, lt, h, :])
                nc.vector.scalar_tensor_tensor(
                    ogh[:, bass.ts(h, 256)], src_o,
                    rv[:, lt, h:h + 1], P["gg"][:, lt, bass.ts(h, 256)],
                    op0=OP.mult, op1=OP.mult)
            if pe_transpose:
                # tail: PE transpose via the idle psT bank beats the
                # ~2.5us DMA-xbar latency on the exposed critical path
                tp = psT.tile([128, 512], BF, tag="tiny", name="tp")
                for q4 in range(4):
                    nc.tensor.matmul(tp[:, bass.ts(q4, 128)],
                                     ogh[:, bass.ts(q4, 128)], ident_b[:],
                                     is_transpose=True, start=True,
                                     stop=True)
                nc.vector.tensor_copy(otT[:, lt, :, :], tp[:])
            else:
                nc.sync.dma_start_transpose(otT[:, lt, :, :], ogh[:])
        for lt in lts:
            ou = small.tile([128, 1024], BF, tag="outsb", bufs=4, name="ou")
            for nh in range(2):
                pop = psA.tile([128, 512], FP, tag="mm512", name="pop")
                for q4 in range(4):
                    nc.tensor.matmul(pop[:], otT[:, lt, q4, :],
                                     wo_t[:, q4, bass.ts(nh, 512)],
                                     start=(q4 == 0), stop=(q4 == 3))
                nc.any.tensor_copy(ou[:, nh * 512:(nh + 1) * 512], pop[:])
            nc.sync.dma_start(
                out[l0 + lt * 128:l0 + (lt + 1) * 128, :], ou[:])

    # ---- main software-pipelined loop --------------------------------------
    Pcur, main0, tail0 = make_prep(0, xt_pre=xt0)
    for f in main0 + tail0:
        f()
    for s in range(NS):
        last = s + 1 >= NS
        if not last:
            Pn, mainN, tailN = make_prep(s + 1)
        else:
            Pn, mainN, tailN = None, [], []
        # distribute prep(s+1) thunks across scan(s)'s chunks:
        # [t_x, beta, q0..3] [k0..3] [v0..3] [tail...]
        groups = [mainN[0:6], mainN[6:10], mainN[10:14], tailN]
        lad = last
        if lad:
            scan_chunk(Pcur, 0, phase="ladder")
        for lt in range(CPS):
            s_mode = 2 if (last and lt == CPS - 1) else 0
            if lad:
                if lt + 1 < CPS:
                    scan_chunk(Pcur, lt + 1, phase="ladder")
                scan_chunk(Pcur, lt, merge_ps=False, s_mode=s_mode,
                           phase="stail")
            else:
                scan_chunk(Pcur, lt, merge_ps=False, s_mode=s_mode)
            for f in groups[lt]:
                f()
            if last:
                # no prep to hide under: flush each chunk as it completes
                rv = small.tile([128, CPS, 2], FP, tag="rv", bufs=1,
                                name="rv")
                nc.scalar.activation(
                    rv[:, lt, :], Pcur["ssqs"][:, lt, :],
                    AF.Abs_reciprocal_sqrt, bias=eps_rms[:], scale=1.0 / Dh)
                flush(Pcur, s, lts=[lt], rv=rv,
                      pe_transpose=True)
        if not last:
            flush(Pcur, s)
        Pcur = Pn


_CACHED_NC = None


def _build():
    global _CACHED_NC
    if _CACHED_NC is not None:
        return _CACHED_NC
    nc = bacc.Bacc("TRN2", target_bir_lowering=False, debug=False)
    io = {}
    io["x"] = nc.dram_tensor("x", [D, L], BF, kind="ExternalInput").ap()
    for nm, shp in (("wq", [D, DL]), ("wk", [D, DL]), ("wv", [D, DL]),
                    ("wg", [D, DL]), ("wb", [D, 2]), ("wo", [DL, D])):
        io[nm] = nc.dram_tensor(nm, shp, BF, kind="ExternalInput").ap()
    for nm in ("cq", "ck", "cv"):
        io[nm] = nc.dram_tensor(nm, [DL, KT], FP, kind="ExternalInput").ap()
    io["out"] = nc.dram_tensor("out", [L, D], BF, kind="ExternalOutput").ap()
    with tile.TileContext(nc) as tc, ExitStack() as ctx:
        deltanet_core(ctx, tc, io)
    nc.compile()
    _CACHED_NC = nc
    return nc


def kernel(hidden_states, Wq, Wk, Wv, Wb, Wg, Wo, conv_q, conv_k, conv_v,
           norm_w):
    bf = mybir.dt.np(BF)
    x = np.asarray(hidden_states, dtype=np.float32)
    Wo_s = np.asarray(Wo, np.float32) * np.tile(np.asarray(norm_w, np.float32),
                                                H)[:, None]
    nc = _build()
    in_maps = []
    for c in range(8):
        b, hg = c // 2, c % 2
        cols = slice(hg * DL, (hg + 1) * DL)
        in_maps.append({
            "x": np.ascontiguousarray(x[b].T).astype(bf),
            "wq": np.asarray(Wq, np.float32)[:, cols].astype(bf),
            "wk": np.asarray(Wk, np.float32)[:, cols].astype(bf),
            "wv": np.asarray(Wv, np.float32)[:, cols].astype(bf),
            "wg": np.asarray(Wg, np.float32)[:, cols].astype(bf),
            "wb": np.asarray(Wb, np.float32)[:, 2 * hg:2 * hg + 2].astype(bf),
            "wo": Wo_s[cols, :].astype(bf),
            "cq": np.ascontiguousarray(np.asarray(conv_q, np.float32)[cols]),
            "ck": np.ascontiguousarray(np.asarray(conv_k, np.float32)[cols]),
            "cv": np.ascontiguousarray(np.asarray(conv_v, np.float32)[cols]),
        })
    res = run_bass_kernel_spmd(nc, in_maps, core_ids=list(range(8)))
    outv = np.zeros((B, L, D), np.float32)
    for c in range(8):
        outv[c // 2] += np.asarray(res.results[c]["out"], np.float32)
    return outv


# revision 106
# speedup vs baseline: 1.0044x; 1.0044x over previous
"""DeltaNet forward Trainium2 kernel (8-core SPMD, batch x head-pair sharded).

Sharding: core c -> batch b=c//2, head-group hg=c%2 (heads 2hg, 2hg+1 = columns
hg*512 : hg*512+512 of the q/k/v/g projections).  Each core computes a partial
[L, D] output (its two heads' contribution through the output projection); the
host sums the two bf16 partials per batch in fp32.  norm_w is folded into Wo
on the host; weights and x are fed to the device in bf16 (host-converted).

Per-core pipeline, software-pipelined across 8 strips of 512 tokens:
  prep(s):  x^T strip DMA (one 1MB transfer) -> q/k/v projections
            z^T = W^T x^T (bf16 matmuls) -> depthwise causal conv = 4
            accumulating diag(w_tap) matmuls reusing the projection's PSUM
            bank -> SiLU; g projection (token-major) with the swish gate
            fused into a single SiLU (o*g*sigmoid(g) = o*silu(g)); all four
            chunks' betas in one batched sigmoid; l2-norm scales via
            ones-matmul column sums + PE row-broadcast; k-hat/q-hat scaled
            copies; token-major k-hat/v via two DMA-xbar transposes each.
  scan(s):  per 128-token chunk, delta-rule scan in bf16; (I+A)^-1 by a
            2-level Neumann product form (A is strongly contractive here);
            A^T/attn^T/A masked+scaled in one 3-part-mask DVE op; only -T is
            materialized (kbtok and U_t carry the compensating signs);
            multi-region PSUM tiles cut the per-chunk allocation count so
            consecutive chunks' ladders overlap; S in fp32 master + bf16
            working copy.
  flush(s): batched RMS-norm + swish gate, one DMA-xbar transpose per chunk,
            output projection -> bf16 partial out [L, D] (4-deep staging so
            the pipe is not paced by out-DMA completion).
  Emission interleaves prep(s+1) into scan(s) chunk-by-chunk; the Tile list
  scheduler uses emission order as priority, so this paces PE-dense prep work
  into the latency-bound scan chain.  Activation-table loads are kept to
  ~3/strip by clustering SiLU / sigmoid / abs-rsqrt uses.
"""

import sys

sys.path.insert(0, "/opt/trn_rl_repo")

from contextlib import ExitStack

import numpy as np

import concourse.bass as bass
import concourse.tile as tile
from concourse import bacc, mybir
from concourse.bass_utils import run_bass_kernel_spmd
from concourse.masks import make_identity

FP = mybir.dt.float32
BF = mybir.dt.bfloat16
AF = mybir.ActivationFunctionType
OP = mybir.AluOpType

B, L, D, H = 4, 4096, 1024, 4
Dh = 256          # head dim
DL = 512          # per-core channels (2 heads)
KT = 4            # conv taps
C = 128           # chunk length
LT = 512          # L-strip size
NS = L // LT      # 8 strips
CPS = LT // C     # 4 chunks per strip
NLEV = 2          # Neumann levels (A is strongly contractive, A^8 ~ 1e-3)
EPS_RMS = 1e-5
EPS_L2 = 1e-12


def deltanet_core(ctx: ExitStack, tc: tile.TileContext, io: dict):
    nc = tc.nc
    x, wq, wk, wv, wg, wb, wo, cq, ck, cv, out = (
        io["x"], io["wq"], io["wk"], io["wv"], io["wg"], io["wb"], io["wo"],
        io["cq"], io["ck"], io["cv"], io["out"])

    pool1 = ctx.enter_context(tc.tile_pool(name="consts", bufs=1))
    xpool = ctx.enter_context(tc.tile_pool(name="xp", bufs=2))
    zpool = ctx.enter_context(tc.tile_pool(name="zp", bufs=3))
    strip = ctx.enter_context(tc.tile_pool(name="strip", bufs=2))
    small = ctx.enter_context(tc.tile_pool(name="small", bufs=2))
    hot = ctx.enter_context(tc.tile_pool(name="hot", bufs=5))
    spool = ctx.enter_context(tc.tile_pool(name="state", bufs=1))
    psA = ctx.enter_context(tc.tile_pool(name="psA", bufs=3, space="PSUM"))

    psS = ctx.enter_context(tc.tile_pool(name="psS", bufs=4, space="PSUM"))
    psT = ctx.enter_context(tc.tile_pool(name="psT", bufs=1, space="PSUM"))



    # ---- first x strip + resident constants (DMA-ordered by first use) -----
    xr0 = x.rearrange("(t p) l -> p t l", p=128)
    xt0 = xpool.tile([128, 8, LT], BF, tag="xt", name="xt0")
    wt = {}
    wt["q"] = pool1.tile([128, 8, DL], BF, tag="wq", name="wq_t")
    wqr = wq.rearrange("(t p) n -> p t n", p=128)
    for quar in range(4):
        hs = slice(2 * quar, 2 * quar + 2)
        nc.sync.dma_start(out=xt0[:, hs, :], in_=xr0[:, hs, 0:LT])
        nc.sync.dma_start(wt["q"][:, hs, :], wqr[:, hs, :])
    for name, w in (("k", wk), ("v", wv)):
        t = pool1.tile([128, 8, DL], BF, tag=f"w{name}", name=f"w{name}")
        nc.sync.dma_start(t[:], w.rearrange("(t p) n -> p t n", p=128))
        wt[name] = t
    cw = {}
    for name, cz in (("q", cq), ("k", ck), ("v", cv)):
        t = pool1.tile([128, 4, KT], FP, tag=f"c{name}", name=f"c{name}")
        nc.sync.dma_start(t[:], cz.rearrange("(t p) j -> p t j", p=128))
        cw[name] = t
    wb_t = pool1.tile([128, 8, 2], BF, tag="wb")
    nc.sync.dma_start(wb_t[:], wb.rearrange("(t p) n -> p t n", p=128))
    t = pool1.tile([128, 8, DL], BF, tag="wg", name="wg_t")
    nc.sync.dma_start(t[:], wg.rearrange("(t p) n -> p t n", p=128))
    wt["g"] = t
    wo_t = pool1.tile([128, 4, 1024], BF, tag="wo")
    nc.sync.dma_start(wo_t[:], wo.rearrange("(t p) n -> p t n", p=128))

    # mask_ua[:, 0, :] strict-upper, mask_ua[:, 1, :] inclusive-upper
    mask3 = pool1.tile([128, 3, 128], BF, tag="mask3")
    nc.gpsimd.memset(mask3, 1.0)
    # parts 0,1: keep where y + a - x > 0 (a=0 strict-upper, a=1 incl-upper)
    nc.gpsimd.affine_select(out=mask3[:, 0:2, :], in_=mask3[:, 0:2, :],
                            compare_op=OP.is_gt, fill=0.0, base=0,
                            pattern=[[1, 2], [1, 128]], channel_multiplier=-1)
    # part 2: strict-lower
    nc.gpsimd.affine_select(out=mask3[:, 2, :], in_=mask3[:, 2, :],
                            compare_op=OP.is_gt, fill=0.0, base=0,
                            pattern=[[-1, 128]], channel_multiplier=1)
    mask_ua = mask3[:, 0:2, :]
    ident_f = pool1.tile([128, 128], FP, tag="identf")
    ident_b = pool1.tile([128, 128], BF, tag="identb")
    make_identity(nc, ident_f)
    make_identity(nc, ident_b)

    ones_col = pool1.tile([128, 1], BF, tag="ones_col")
    nc.vector.memset(ones_col, 1.0)
    # sel24[:, r, :] is e_r (x) ones: lhsT for K=24 row-broadcast matmuls
    sel24 = pool1.tile([24, 24, 128], BF, tag="sel24")
    nc.gpsimd.memset(sel24, 1.0)
    nc.gpsimd.affine_select(out=sel24[:], in_=sel24[:], compare_op=OP.is_equal,
                            fill=0.0, base=0, pattern=[[-1, 24], [0, 128]],
                            channel_multiplier=1)
    eps_l2 = pool1.tile([128, 1], FP, tag="epsl2")
    nc.vector.memset(eps_l2, EPS_L2 * EPS_L2)
    eps_rms = pool1.tile([128, 1], FP, tag="epsrms")
    nc.vector.memset(eps_rms, EPS_RMS)
    sq_scr = pool1.tile([128, 256], BF, tag="sq_scr")

    # conv diag tiles (bf16)
    diag = {}
    for name in ("q", "k", "v"):
        dt = pool1.tile([128, 4, KT, 128], BF, tag=f"diag{name}")
        diag[name] = dt
        for ct in range(4):
            for j in range(KT):
                nc.vector.tensor_scalar_mul(dt[:, ct, j, :], ident_b[:],
                                            cw[name][:, ct, j:j + 1])

    # ---- persistent state ---------------------------------------------------
    S32 = [spool.tile([128, 2, 256], FP, name=f"S32_{h}", tag=f"S32_{h}")
           for h in range(2)]
    Sbf = [spool.tile([128, 2, 256], BF, name=f"Sbf_{h}", tag=f"Sbf_{h}")
           for h in range(2)]
    for h in range(2):
        nc.vector.memset(S32[h], 0.0)
        nc.vector.memset(Sbf[h], 0.0)
    carries = {}
    for name in ("q", "k", "v"):
        for ct in range(4):
            cr = spool.tile([128, KT - 1], BF, tag=f"carry_{name}{ct}",
                            name="cr")
            nc.vector.memset(cr, 0.0)
            carries[(name, ct)] = cr

    xr = x.rearrange("(t p) l -> p t l", p=128)


    # ---- per-strip emission helpers ----------------------------------------
    def make_prep(s, xt_pre=None):
        """Returns (P, main_thunks, tail_thunks) for strip s."""
        P = {}
        l0 = s * LT

        def t_x():
            if xt_pre is not None:
                xt = xt_pre
            else:
                xt = xpool.tile([128, 8, LT], BF, tag="xt", name="xt")
                nc.sync.dma_start(out=xt[:], in_=xr[:, :, l0:l0 + LT])
            P["xt"] = xt
            P["ssqs"] = small.tile([128, CPS, 2], FP, tag="ssqs", name="ssqs")
            P["vtok"] = strip.tile([128, 2, 2, CPS, 128], BF, tag="vtok",
                                   name="vtok")
            P["ktok"] = strip.tile([128, 2, 2, CPS, 128], BF, tag="ktok",
                                   name="ktok")
            P["gg"] = strip.tile([128, CPS, DL], BF, tag="gg", name="gg")
            P["osb"] = strip.tile([128, CPS, 2, 256], BF, tag="osb",
                                  name="osb")

        def mk_qkv(name, ct):
            def f():
                xt = P["xt"]
                zp_ = psA.tile([128, LT], FP, tag="mm512", name="zp")
                for kt8 in range(8):
                    nc.tensor.matmul(
                        zp_[:], wt[name][:, kt8, bass.ts(ct, 128)],
                        xt[:, kt8, :], start=(kt8 == 0), stop=(kt8 == 7))
                ze = zpool.tile([128, KT - 1 + LT], BF, tag="zext", name="ze")
                nc.gpsimd.tensor_copy(ze[:, 0:KT - 1], carries[(name, ct)][:])
                nc.any.tensor_copy(ze[:, KT - 1:], zp_[:])
                nc.gpsimd.tensor_copy(carries[(name, ct)][:],
                                      ze[:, LT:LT + KT - 1])
                zc = zp_
                for j in range(KT):
                    nc.tensor.matmul(zc[:], diag[name][:, ct, j, :],
                                     ze[:, j:j + LT],
                                     start=(j == 0), stop=(j == KT - 1))
                if name == "v":
                    h, ct2 = divmod(ct, 2)
                    if ("zsv", h) not in P:
                        P[("zsv", h)] = strip.tile([128, 2, LT], BF,
                                                   tag=f"zsv{h}", bufs=1,
                                                   name="zsv")
                    zst = P[("zsv", h)][:, ct2, :]
                    nc.scalar.activation(zst, zc[:], AF.Silu)
                    if ct2 == 1:
                        nc.sync.dma_start_transpose(P["vtok"][:, h],
                                                    P[("zsv", h)][:])
                else:
                    zst = strip.tile([128, LT], BF, tag=f"zs_{name}{ct}",
                                     bufs=1, name="zst")
                    nc.scalar.activation(zst[:], zc[:], AF.Silu)
                    P[("zs", name, ct)] = zst
                    sqt = strip.tile([128, LT], BF, tag=f"sq_{name}{ct}",
                                     bufs=1, name="sqt")
                    nc.gpsimd.tensor_tensor(sqt[:], zst[:], zst[:],
                                            op=OP.mult)
                    P[("sq", name, ct)] = sqt
            return f

        def mk_g(lt):
            def f():
                xt = P["xt"]
                gp = psA.tile([128, 512], FP, tag="mm512", name="gp")
                for kt8 in range(8):
                    nc.tensor.matmul(gp[:], xt[:, kt8, bass.ts(lt, 128)],
                                     wt["g"][:, kt8, :],
                                     start=(kt8 == 0), stop=(kt8 == 7))
                nc.scalar.activation(P["gg"][:, lt, :], gp[:], AF.Silu)
            return f

        def t_beta():
            xt = P["xt"]
            bp = psT.tile([128, 512], FP, tag="tiny", name="bp")
            for lt in range(CPS):
                for kt8 in range(8):
                    nc.tensor.matmul(bp[:, 2 * lt:2 * lt + 2],
                                     xt[:, kt8, bass.ts(lt, 128)],
                                     wb_t[:, kt8, :],
                                     start=(kt8 == 0), stop=(kt8 == 7))
            betas = small.tile([128, CPS, 2], FP, tag="betas", name="betas")
            nc.scalar.activation(betas[:].rearrange("p l h -> p (l h)"),
                                 bp[:, 0:8], AF.Sigmoid)
            P["betas"] = betas

        def t_psq():
            # l2 column sums + rsq scales; rsqall cols: ni*8+h*4+lt for
            # rsq_{q,k}; 16+h*4+lt for rsq_k * beta
            psq = psT.tile([128, 512], FP, tag="tiny", name="psq")
            for ni, name in enumerate(("q", "k")):
                for h in range(2):
                    for lt in range(CPS):
                        col = ni * 8 + h * 4 + lt
                        for ct2 in range(2):
                            nc.tensor.matmul(
                                psq[:, col:col + 1],
                                P[("sq", name, 2 * h + ct2)][:,
                                                             bass.ts(lt, 128)],
                                ones_col[:], start=(ct2 == 0),
                                stop=(ct2 == 1))
            rsqall = small.tile([128, 24], FP, tag="rsqall", name="rsqall")
            nc.scalar.activation(rsqall[:, 0:16], psq[:, 0:16],
                                 AF.Abs_reciprocal_sqrt, bias=eps_l2[:])
            nc.vector.tensor_tensor(
                rsqall[:, 16:24].rearrange("p (h l) -> p h l", h=2),
                rsqall[:, 8:16].rearrange("p (h l) -> p h l", h=2),
                P["betas"].rearrange("p l h -> p h l"), op=OP.mult)
            rpt = psq
            nc.tensor.matmul(rpt[0:24, 128:256], rsqall[:], ident_f[:],
                             is_transpose=True, start=True, stop=True)
            rr24 = small.tile([24, 128], BF, tag="rr24", name="rr24")
            nc.scalar.copy(rr24[:], rpt[0:24, 128:256])
            P["rr24"] = rr24

        def mk_bcast(h):
            def f():
                bc = strip.tile([128, CPS, 3, 128], BF, tag=f"bcast{h}",
                                bufs=1, name="bc")
                for lt in range(CPS):
                    bp2 = psA.tile([128, 3 * 128], FP, tag="mm512",
                                   name="bp2")
                    for r, row in enumerate((h * 4 + lt, 8 + h * 4 + lt,
                                             16 + h * 4 + lt)):
                        nc.tensor.matmul(bp2[:, bass.ts(r, 128)],
                                         sel24[:, row, :], P["rr24"][:],
                                         start=True, stop=True)
                    nc.any.tensor_copy(bc[:, lt, :, :], bp2[:])
                P[("bc", h)] = bc
            return f

        def mk_kqkh(h, ct2):
            def f():
                ct = 2 * h + ct2
                bc = P[("bc", h)]

                def c4(ap):
                    return ap.rearrange("p (a b) -> p a b", a=CPS)

                kq = strip.tile([128, 2, LT], BF, tag=f"kqT{h}{ct2}",
                                name="kq")
                nc.vector.tensor_tensor(c4(kq[:, 0, :]),
                                        c4(P[("zs", "k", ct)][:]),
                                        bc[:, :, 2, :], op=OP.mult)
                nc.vector.tensor_tensor(c4(kq[:, 1, :]),
                                        c4(P[("zs", "q", ct)][:]),
                                        bc[:, :, 0, :], op=OP.mult)
                if ("khT", h) not in P:
                    P[("khT", h)] = strip.tile([128, 2, LT], BF,
                                               tag=f"khT{h}", name="khh")
                kh = P[("khT", h)][:, ct2, :]
                nc.vector.tensor_tensor(c4(kh), c4(P[("zs", "k", ct)][:]),
                                        bc[:, :, 1, :], op=OP.mult)
                P[("kqT", h, ct2)] = kq
                if ct2 == 1:
                    nc.sync.dma_start_transpose(P["ktok"][:, h],
                                                P[("khT", h)][:])
            return f

        main = [t_x, t_beta]
        for name in ("q", "k", "v"):
            for ct in range(4):
                main.append(mk_qkv(name, ct))
        tail = [t_psq, mk_bcast(0), mk_bcast(1)]
        for h in range(2):
            for ct2 in range(2):
                tail.append(mk_kqkh(h, ct2))
        for lt in range(CPS):
            tail.append(mk_g(lt))
        return P, main, tail

    def scan_chunk(P, lt, merge_ps=False, s_mode=0):
        betas = P["betas"]
        ktok, vtok = P["ktok"], P["vtok"]
        cs = bass.ts(lt, 128)
        # The two heads' scans are emitted stage-interleaved: engines execute
        # their streams in order, so alternating heads gives each head's
        # mm->copy->mm ladder a gap-filler.
        vb, kbtok, ATat, Alow = {}, {}, {}, {}
        R, Lk, Uk, negT, Ut, pos = {}, {}, {}, {}, {}, {}
        p1, p2, pp, pw, pu = {}, {}, {}, {}, {}
        for h in range(2):
            bcol = betas[:, lt, h:h + 1]
            vb[h] = hot.tile([128, 2, 128], BF, tag="vb", name=f"vb{h}")
            nc.gpsimd.tensor_scalar_mul(vb[h][:], vtok[:, h, :, lt, :], bcol)
            kbtok[h] = hot.tile([128, 2, 128], BF, tag="kbtok",
                                name=f"kbtok{h}")
            nc.gpsimd.tensor_scalar(kbtok[h][:], ktok[:, h, :, lt, :],
                                    bcol, -1.0, op0=OP.mult, op1=OP.mult)
        for h in range(2):
            # T1 regions: p1 (A^T|attn^T) 0:256, p2 (A) 256:384,
            # lev-1 R-update 384:512
            p1[h] = psS.tile([128, 512], FP, tag="scan", name="p1")
            for ct2 in range(2):
                nc.tensor.matmul(p1[h][:, 0:256],
                                 P[("khT", h)][:, ct2, cs],
                                 P[("kqT", h, ct2)][:, :, cs],
                                 start=(ct2 == 0), stop=(ct2 == 1))
            p2[h] = p1[h]
            for ct2 in range(2):
                nc.tensor.matmul(p2[h][:, 256:384],
                                 P[("kqT", h, ct2)][:, 0, cs],
                                 P[("khT", h)][:, ct2, cs],
                                 start=(ct2 == 0), stop=(ct2 == 1))
        for h in range(2):
            AAl = hot.tile([128, 384], BF, tag="ATat", name=f"AAl{h}")
            nc.vector.tensor_tensor(
                AAl[:].rearrange("p (a b) -> p a b", a=3),
                p1[h][:, 0:384].rearrange("p (a b) -> p a b", a=3),
                mask3[:], op=OP.mult)
            ATat[h] = AAl[:, 0:256]
            Alow[h] = AAl[:, 256:384]
            R[h] = hot.tile([128, 128], BF, tag="Rn", name=f"R{h}")
            nc.gpsimd.tensor_tensor(R[h][:], ident_b[:], AAl[:, 0:128],
                                    op=OP.subtract)
            Lk[h] = Alow[h]
            Uk[h] = ATat[h][:, 0:128]
        # Neumann (NLEV=2): (I - A_T)(I + A_T^2)(I + A_T^4).
        # T2 regions: lev0 UL/LU 0:256, lev1 UL/LU 256:512, pw -> 0:128
        # (reused after lev0 copy); lev1 R-update -> T1 384:512.
        for h in range(2):
            pp[h] = psS.tile([128, 512], FP, tag="scan", name="pp")
            nc.tensor.matmul(pp[h][:, 0:128], Uk[h], Lk[h],
                             start=True, stop=True)
            nc.tensor.matmul(pp[h][:, 128:256], Lk[h], Uk[h],
                             start=True, stop=True)
        for h in range(2):
            LUR = hot.tile([128, 384], BF, tag="LUR", name=f"LUR{h}")
            nc.any.tensor_copy(LUR[:, 0:256], pp[h][:, 0:256])
            Lk[h] = LUR[:, 0:128]
            Uk[h] = LUR[:, 128:256]
        for h in range(2):
            nc.tensor.matmul(pp[h][:, 256:384], Uk[h], Lk[h],
                             start=True, stop=True)
            nc.tensor.matmul(pp[h][:, 384:512], Lk[h], Uk[h],
                             start=True, stop=True)
            nc.tensor.matmul(p1[h][:, 384:512], Lk[h], R[h][:],
                             start=True, stop=False)
            nc.tensor.matmul(p1[h][:, 384:512], ident_b[:], R[h][:],
                             start=False, stop=True)
        for h in range(2):
            LUR = hot.tile([128, 384], BF, tag="LUR", name=f"LUR{h}b")
            nc.any.tensor_copy(LUR[:, 0:256], pp[h][:, 256:512])
            nc.any.tensor_copy(LUR[:, 256:384], p1[h][:, 384:512])
            R[h] = LUR[:, 256:384]
            Lk[h] = LUR[:, 0:128]
            Uk[h] = LUR[:, 128:256]
        for h in range(2):
            # final factor: R <- (I + A_T^4) R, into T2 0:128 (freed)
            pw[h] = pp[h]
            nc.tensor.matmul(pw[h][:, 0:128], Lk[h], R[h][:],
                             start=True, stop=False)
            nc.tensor.matmul(pw[h][:, 0:128], ident_b[:], R[h][:],
                             start=False, stop=True)
        for h in range(2):
            negT[h] = hot.tile([128, 128], BF, tag="negT", name=f"negT{h}")
            nc.vector.tensor_scalar_mul(negT[h][:], pw[h][:, 0:128], -1.0)
        for h in range(2):
            pu[h] = psS.tile([128, 512], FP, tag="scan", name="pu")
            for half in range(2):
                nc.tensor.matmul(pu[h][:, bass.ts(half, 128)],
                                 kbtok[h][:, half, :], negT[h][:],
                                 start=True, stop=True)
        WT = {}
        for h in range(2):
            WT[h] = hot.tile([128, 2, 128], BF, tag="WT", name=f"WT{h}")
            nc.any.tensor_copy(WT[h][:], pu[h][:, 0:256])
        for h in range(2):
            # -U = (-T) vb + W S (accumulated in psum); Ut negates on copy
            nc.tensor.matmul(pu[h][:, 256:512], negT[h][:],
                             vb[h][:].rearrange("p a b -> p (a b)"),
                             start=True, stop=False)
            for half in range(2):
                nc.tensor.matmul(pu[h][:, 256:512], WT[h][:, half, :],
                                 Sbf[h][:, half, :],
                                 start=False, stop=(half == 1))
        for h in range(2):
            Ut[h] = hot.tile([128, 256], BF, tag="Ut", name=f"Ut{h}")
            nc.vector.tensor_scalar_mul(Ut[h][:], pu[h][:, 256:512], -1.0)
        for h in range(2):
            if merge_ps:
                po = pp[h]
            else:
                po = psS.tile([128, 512], FP, tag="scan", name="po")
            for half in range(2):
                nc.tensor.matmul(po[:, 0:256], P[("kqT", h, half)][:, 1, cs],
                                 Sbf[h][:, half, :],
                                 start=(half == 0), stop=False)
            nc.tensor.matmul(po[:, 0:256], ATat[h][:, 128:256], Ut[h][:],
                             start=False, stop=True)
            pos[h] = po
            if s_mode != 2:
                if merge_ps:
                    psu = p1[h]
                else:
                    psu = psS.tile([128, 512], FP, tag="scan", name="psu")
                for half in range(2):
                    nc.tensor.matmul(psu[:, bass.ts(half, 256)],
                                     ktok[:, h, half, lt, :], Ut[h][:],
                                     start=True, stop=True)
            if s_mode == 0:
                nc.vector.tensor_tensor(
                    S32[h][:].rearrange("p a b -> p (a b)"), psu[:, 0:512],
                    S32[h][:].rearrange("p a b -> p (a b)"), op=OP.add)
                nc.gpsimd.tensor_copy(Sbf[h][:], S32[h][:])
            elif s_mode == 1:
                # tail strip: Sbf straight from psu + S32_old (short chain);
                # fp32 master updated in parallel for the next chunk's read
                nc.vector.scalar_tensor_tensor(
                    Sbf[h][:].rearrange("p a b -> p (a b)"), psu[:, 0:512],
                    1.0, S32[h][:].rearrange("p a b -> p (a b)"),
                    op0=OP.mult, op1=OP.add)
                nc.vector.tensor_tensor(
                    S32[h][:].rearrange("p a b -> p (a b)"), psu[:, 0:512],
                    S32[h][:].rearrange("p a b -> p (a b)"), op=OP.add)
            # o to SBUF + sum of squares for the batched RMS norm
            nc.any.tensor_copy(P["osb"][:, lt, h, :], po[:, 0:256])
            nc.scalar.activation(sq_scr[:], po[:, 0:256], AF.Square,
                                 accum_out=P["ssqs"][:, lt, h:h + 1])

    def flush(P, s, lts=None, rv=None, pe_transpose=False):
        l0 = s * LT
        lts = range(CPS) if lts is None else lts
        if rv is None:
            rv = small.tile([128, CPS, 2], FP, tag="rv", bufs=1, name="rv")
            nc.scalar.activation(rv[:].rearrange("p a b -> p (a b)"),
                                 P["ssqs"][:].rearrange("p a b -> p (a b)"),
                                 AF.Abs_reciprocal_sqrt, bias=eps_rms[:],
                                 scale=1.0 / Dh)
        if "otT" not in P:
            P["otT"] = strip.tile([128, CPS, 4, 128], BF, tag="otT",
                                  name="otT")
        otT = P["otT"]
        for lt in lts:
            ogh = hot.tile([128, 512], BF, tag="ogh", bufs=3, name="ogh")
            for h in range(2):
                nc.vector.scalar_tensor_tensor(
                    ogh[:, bass.ts(h, 256)], P["osb"][:, lt, h, :],
                    rv[:, lt, h:h + 1], P["gg"][:, lt, bass.ts(h, 256)],
                    op0=OP.mult, op1=OP.mult)
            if pe_transpose:
                # tail: PE transpose via the idle psT bank beats the
                # ~2.5us DMA-xbar latency on the exposed critical path
                tp = psT.tile([128, 512], BF, tag="tiny", name="tp")
                for q4 in range(4):
                    nc.tensor.matmul(tp[:, bass.ts(q4, 128)],
                                     ogh[:, bass.ts(q4, 128)], ident_b[:],
                                     is_transpose=True, start=True,
                                     stop=True)
                nc.vector.tensor_copy(otT[:, lt, :, :], tp[:])
            else:
                nc.sync.dma_start_transpose(otT[:, lt, :, :], ogh[:])
        for lt in lts:
            ou = small.tile([128, 1024], BF, tag="outsb", bufs=4, name="ou")
            for nh in range(2):
                pop = psA.tile([128, 512], FP, tag="mm512", name="pop")
                for q4 in range(4):
                    nc.tensor.matmul(pop[:], otT[:, lt, q4, :],
                                     wo_t[:, q4, bass.ts(nh, 512)],
                                     start=(q4 == 0), stop=(q4 == 3))
                nc.any.tensor_copy(ou[:, nh * 512:(nh + 1) * 512], pop[:])
            nc.sync.dma_start(
                out[l0 + lt * 128:l0 + (lt + 1) * 128, :], ou[:])

    # ---- main software-pipelined loop --------------------------------------
    Pcur, main0, tail0 = make_prep(0, xt_pre=xt0)
    for f in main0 + tail0:
        f()
    for s in range(NS):
        last = s + 1 >= NS
        if not last:
            Pn, mainN, tailN = make_prep(s + 1)
        else:
            Pn, mainN, tailN = None, [], []
        # distribute prep(s+1) thunks across scan(s)'s chunks:
        # [t_x, beta, q0..3] [k0..3] [v0..3] [tail...]
        groups = [mainN[0:6], mainN[6:10], mainN[10:14], tailN]
        lad = last
        if lad:
            scan_chunk(Pcur, 0, phase="ladder")
        for lt in range(CPS):
            s_mode = 2 if (last and lt == CPS - 1) else 0
            if lad:
                if lt + 1 < CPS:
                    scan_chunk(Pcur, lt + 1, phase="ladder")
                scan_chunk(Pcur, lt, merge_ps=False, s_mode=s_mode,
                           phase="stail")
            else:
                scan_chunk(Pcur, lt, merge_ps=False, s_mode=s_mode)
            for f in groups[lt]:
                f()
            if last:
                # no prep to hide under: flush each chunk as it completes
                rv = small.tile([128, CPS, 2], FP, tag="rv", bufs=1,
                                name="rv")
                nc.scalar.activation(
                    rv[:, lt, :], Pcur["ssqs"][:, lt, :],
                    AF.Abs_reciprocal_sqrt, bias=eps_rms[:], scale=1.0 / Dh)
                flush(Pcur, s, lts=[lt], rv=rv,
                      pe_transpose=True)
        if not last:
            flush(Pcur, s)
        Pcur = Pn


_CACHED_NC = None


def _build():
    global _CACHED_NC
    if _CACHED_NC is not None:
        return _CACHED_NC
    nc = bacc.Bacc("TRN2", target_bir_lowering=False, debug=False)
    io = {}
    io["x"] = nc.dram_tensor("x", [D, L], BF, kind="ExternalInput").ap()
    for nm, shp in (("wq", [D, DL]), ("wk", [D, DL]), ("wv", [D, DL]),
                    ("wg", [D, DL]), ("wb", [D, 2]), ("wo", [DL, D])):
        io[nm] = nc.dram_tensor(nm, shp, BF, kind="ExternalInput").ap()
    for nm in ("cq", "ck", "cv"):
        io[nm] = nc.dram_tensor(nm, [DL, KT], FP, kind="ExternalInput").ap()
    io["out"] = nc.dram_tensor("out", [L, D], BF, kind="ExternalOutput").ap()
    with tile.TileContext(nc) as tc, ExitStack() as ctx:
        deltanet_core(ctx, tc, io)
    nc.compile()
    _CACHED_NC = nc
    return nc


def kernel(hidden_states, Wq, Wk, Wv, Wb, Wg, Wo, conv_q, conv_k, conv_v,
           norm_w):
    bf = mybir.dt.np(BF)
    x = np.asarray(hidden_states, dtype=np.float32)
    Wo_s = np.asarray(Wo, np.float32) * np.tile(np.asarray(norm_w, np.float32),
                                                H)[:, None]
    nc = _build()
    in_maps = []
    for c in range(8):
        b, hg = c // 2, c % 2
        cols = slice(hg * DL, (hg + 1) * DL)
        in_maps.append({
            "x": np.ascontiguousarray(x[b].T).astype(bf),
            "wq": np.asarray(Wq, np.float32)[:, cols].astype(bf),
            "wk": np.asarray(Wk, np.float32)[:, cols].astype(bf),
            "wv": np.asarray(Wv, np.float32)[:, cols].astype(bf),
            "wg": np.asarray(Wg, np.float32)[:, cols].astype(bf),
            "wb": np.asarray(Wb, np.float32)[:, 2 * hg:2 * hg + 2].astype(bf),
            "wo": Wo_s[cols, :].astype(bf),
            "cq": np.ascontiguousarray(np.asarray(conv_q, np.float32)[cols]),
            "ck": np.ascontiguousarray(np.asarray(conv_k, np.float32)[cols]),
            "cv": np.ascontiguousarray(np.asarray(conv_v, np.float32)[cols]),
        })
    res = run_bass_kernel_spmd(nc, in_maps, core_ids=list(range(8)))
    outv = np.zeros((B, L, D), np.float32)
    for c in range(8):
        outv[c // 2] += np.asarray(res.results[c]["out"], np.float32)
    return outv
